# revision 10
# baseline (speedup 1.0000x reference)
"""Trainium2 Bass kernel for nn_CapsNet_69114613730132.

Strategy (8 NeuronCores, SPMD, zero collectives):
  The CapsNet routing loop is degenerate (self.bij is never updated, so
  cij stays 1/512) and collapses to: conv1 -> conv2 -> squash ->
  4096->160 matvec -> elementwise squash. The convolutions are tiny, so
  cross-core collectives (AllGather floor + a ~40us rank-alignment
  barrier measured on this fabric) cost more than replicating them.

  * Every core computes conv1 + conv2 (PrimaryCaps) + squash redundantly:
      conv1 as a 243-contraction matmul over a host-built im2col of x;
      conv2 as 81 (dy,dx) PSUM-accumulated matmuls over strided views of
      h (no im2col materialization), weights stationary, bf16.
  * The DigitCaps matvec output (160 = 10*16) is sharded 20-per-core via
    per-core weight slices => cores are fully independent; the host just
    concatenates the 8 (1,20) results. No communication at all.
  * All PE compute in bf16 (weights host-cast), f32 PSUM/vector math.

kernel(**inputs) takes the FULL unsharded inputs and returns the full
(1,1,10,16,1) float32 output.
"""
import numpy as np
import ml_dtypes

import concourse.bass as bass
import concourse.bacc as bacc
import concourse.tile as tile
import concourse.mybir as mybir
from concourse.bass_utils import run_bass_kernel_spmd
from concourse.tile import ScopedClock

FAST_TAIL = True


class FastTailTileContext(tile.TileContext):
    """TileContext whose kernel-tail barriers use the cheap sequencer-level
    (sem-only) all-engine barrier instead of the drain+EVSEM butterfly
    (~3-4us per barrier on HW). The sync.drain still waits for every
    tracked semaphore target, so all sem-touching work has retired before
    the clears; re-execution safety is preserved by the barriers."""

    def _drain_and_barrier(self, tick_clock, wait_clock):
        if not FAST_TAIL:
            return super()._drain_and_barrier(tick_clock, wait_clock)
        nc = self.nc
        drain_inst = nc.sync.drain()
        wait_clock.add_sem_waits(
            drain_inst.ins, ScopedClock({None: tick_clock.global_clock})
        )
        nc.all_engine_barrier(sem_only=True)
        popped = nc._tile_sem_poison_stack.pop()
        assert popped is self._sem_poison
        nc.clear_and_free_semaphores(list(self.sems.allocated().values()))
        nc.all_engine_barrier(sem_only=True)

BF16 = ml_dtypes.bfloat16
F32 = mybir.dt.float32
BF = mybir.dt.bfloat16

NCORES = 8
KI = 20             # digitcaps output elems per core (160 = 8*20)
W2CHUNK = 9         # dydx positions per w2 DMA chunk (81 = 9*9)


# --------------------------------------------------------------------------
# Host-side input marshalling (pure layout transforms + dtype casts)
# --------------------------------------------------------------------------

def _host_prep(x, conv_w, conv_b, pri_w, pri_b, W):
    x = np.asarray(x, np.float32)
    conv_w = np.asarray(conv_w, np.float32)
    conv_b = np.asarray(conv_b, np.float32)
    pri_w = np.asarray(pri_w, np.float32)
    pri_b = np.asarray(pri_b, np.float32)
    W = np.asarray(W, np.float32)

    # im2col of x: (243, 256), row (c,dy,dx), col (oy*16+ox)
    im2col1 = np.empty((3, 9, 9, 16, 16), np.float32)
    for dy in range(9):
        for dx in range(9):
            im2col1[:, dy, dx] = x[0, :, dy:dy + 16, dx:dx + 16]
    im2col1 = im2col1.reshape(243, 256).astype(BF16)

    W1T = conv_w.reshape(128, 243).T.astype(BF16)  # (243, 128)

    # (ic, dydx*256 + oc2) with oc2 = cap*8 + j
    w2s = (pri_w.reshape(256, 128, 9, 9)
           .transpose(2, 3, 1, 0)          # (dy, dx, ic, oc2)
           .reshape(81, 128, 256)
           .transpose(1, 0, 2)             # (ic, dydx, oc2)
           .reshape(128, 81 * 256).astype(BF16))

    # digitcaps weights V[h, s, p, ki]:
    #   oc2 = 128h+p; cap=oc2>>3; j=oc2&7; n = cap*16 + j*2 + (s>>3); jj = s&7
    Wd = W[0]  # (512, 10, 16, 8)
    oc2 = np.arange(256)
    n_base = (oc2 >> 3) * 16 + (oc2 & 7) * 2
    V = np.empty((2, 16, 128, 160), np.float32)
    for s in range(16):
        sel = Wd[n_base + (s >> 3), :, :, s & 7]      # (256, 10, 16)
        V[:, s] = sel.reshape(2, 128, 160)

    pb2 = pri_b.reshape(2, 128).T.copy()  # (128, 2) [p, h]
    cb2 = conv_b.reshape(128, 1)

    c1 = np.concatenate([im2col1, W1T], axis=1)  # (243, 384) = [im2col | w1t]
    shared = {
        "c1_a": np.ascontiguousarray(c1[:128]),
        "c1_b": np.ascontiguousarray(c1[128:]),
        "w2s": w2s,
        "cb": cb2,
        "pb": pb2,
    }
    per_core = []
    for c in range(NCORES):
        vsl = V[:, :, :, c * KI:(c + 1) * KI]                     # (2,16,128,20)
        vsl = vsl.transpose(2, 0, 1, 3).reshape(128, 32 * KI)     # (128, 640)
        d = dict(shared)
        d["v"] = np.ascontiguousarray(vsl).astype(BF16)
        per_core.append(d)
    return per_core


INPUT_SPECS = {
    "c1_a": ((128, 384), BF),
    "c1_b": ((115, 384), BF),
    "w2s": ((128, 81 * 256), BF),
    "v": ((128, 32 * KI), BF),
    "cb": ((128, 1), F32),
    "pb": ((128, 2), F32),
}


# --------------------------------------------------------------------------
# Device IR
# --------------------------------------------------------------------------

def emit_kernel(tc, out_ap, ins):
    nc = tc.nc
    nw2 = 81 // W2CHUNK
    with (
        tc.tile_pool(name="sb", bufs=1) as sb,
        tc.tile_pool(name="ps", bufs=1, space="PSUM") as ps,
    ):
        # ---- conv1 inputs first, then the w2 stream, on the sync HWDGE
        # ring (FIFO). w2 tiles double-buffer (bufs=2) so at most two DMAs
        # are in flight: completion semaphores then fire per-chunk instead
        # of smearing across the whole interleaved stream.
        c1_a_sb = sb.tile([128, 384], BF)
        c1_b_sb = sb.tile([115, 384], BF)
        nc.sync.dma_start(c1_a_sb[:], ins["c1_a"][:])
        nc.sync.dma_start(c1_b_sb[:], ins["c1_b"][:])
        w2t = []
        for j in range(nw2):
            wt = sb.tile([128, W2CHUNK * 256], BF, name=f"w2t{j}",
                         tag="w2", bufs=2)
            nc.sync.dma_start(
                wt[:], ins["w2s"][:, j * W2CHUNK * 256:(j + 1) * W2CHUNK * 256])
            w2t.append(wt)
        # small/late inputs on the scalar HWDGE ring (parallel)
        cb_sb = sb.tile([128, 1], F32)
        pb_sb = sb.tile([128, 2], F32)
        v_sb = sb.tile([128, 32 * KI], BF)
        for t, name in ((cb_sb, "cb"), (pb_sb, "pb"), (v_sb, "v")):
            nc.scalar.dma_start(t[:], ins[name][:])

        # ---- conv1: h = W1T.T @ im2col + conv_b  -> (128, 256) bf16
        psum1 = ps.tile([128, 256], F32)
        nc.tensor.matmul(psum1[:], c1_a_sb[:, 256:384], c1_a_sb[:, 0:256],
                         start=True, stop=False)
        nc.tensor.matmul(psum1[:], c1_b_sb[:, 256:384], c1_b_sb[:, 0:256],
                         start=False, stop=True)
        h_sb = sb.tile([128, 256], BF)
        nc.vector.tensor_scalar_add(h_sb[:], psum1[:], cb_sb[:])
        h4 = h_sb[:].rearrange("p (y x) -> p y x", y=16)

        # ---- conv2: 81 strided-view matmuls per oc2-half, PSUM-accumulated
        psum2a = ps.tile([128, 16], F32)
        psum2b = ps.tile([128, 16], F32)
        halves = (psum2a, psum2b)
        for dydx in range(81):
            dy, dx = divmod(dydx, 9)
            j, jj = divmod(dydx, W2CHUNK)
            rhs = h4[:, dy:dy + 8:2, dx:dx + 8:2]
            for hh in range(2):
                nc.tensor.matmul(
                    halves[hh][:],
                    w2t[j][:, jj * 256 + hh * 128: jj * 256 + (hh + 1) * 128],
                    rhs,
                    start=(dydx == 0), stop=(dydx == 80),
                )

        # ---- + pri_b -> x2b (128, 32) [p, h*16+s]
        x2b = sb.tile([128, 32], F32)
        for hh in range(2):
            nc.vector.tensor_scalar_add(
                x2b[:, hh * 16:(hh + 1) * 16], halves[hh][:], pb_sb[:, hh:hh + 1])

        # ---- squash factors per (p, h, s_hi) group of 8
        # f = sqrt(sq)/512 / (1+sq)   (1/512 cij folded in)
        t2 = sb.tile([128, 32], F32)
        nc.vector.tensor_mul(t2[:], x2b[:], x2b[:])
        sq = sb.tile([128, 4], F32)
        nc.vector.tensor_reduce(
            sq[:], t2[:].rearrange("p (g e) -> p g e", e=8),
            axis=mybir.AxisListType.X, op=mybir.AluOpType.add,
        )
        r_ = sb.tile([128, 4], F32)
        nc.scalar.activation(
            r_[:], sq[:], mybir.ActivationFunctionType.Sqrt,
            scale=1.0 / (512.0 * 512.0),
        )
        d2 = sb.tile([128, 4], F32)
        nc.vector.tensor_scalar_add(d2[:], sq[:], 1.0)
        rec2 = sb.tile([128, 4], F32)
        nc.vector.reciprocal(rec2[:], d2[:])
        f_ = sb.tile([128, 4], F32)
        nc.vector.tensor_mul(f_[:], r_[:], rec2[:])

        u_sb = sb.tile([128, 32], BF)
        nc.vector.tensor_mul(
            u_sb[:].rearrange("p (g e) -> p g e", e=8),
            x2b[:].rearrange("p (g e) -> p g e", e=8),
            f_[:].broadcast_to((128, 4, 8)),
        )

        # ---- digitcaps matvec: psum_d[0, ki] = sum_{h,s,p} u * V
        psum_d = ps.tile([1, KI], F32)
        for idx in range(32):
            nc.tensor.matmul(
                psum_d[:],
                u_sb[:, idx:idx + 1],
                v_sb[:, idx * KI:(idx + 1) * KI],
                start=(idx == 0), stop=(idx == 31),
            )

        # ---- final elementwise squash: vij = s*|s|/(1+s^2)
        s_sb = sb.tile([1, KI], F32)
        nc.vector.tensor_copy(s_sb[:], psum_d[:])
        t3 = sb.tile([1, KI], F32)
        nc.vector.tensor_mul(t3[:], s_sb[:], s_sb[:])
        d3 = sb.tile([1, KI], F32)
        nc.vector.tensor_scalar_add(d3[:], t3[:], 1.0)
        rec3 = sb.tile([1, KI], F32)
        nc.vector.reciprocal(rec3[:], d3[:])
        a3 = sb.tile([1, KI], F32)
        nc.scalar.activation(a3[:], t3[:], mybir.ActivationFunctionType.Sqrt)
        m3 = sb.tile([1, KI], F32)
        nc.vector.tensor_mul(m3[:], a3[:], s_sb[:])
        o3 = sb.tile([1, KI], F32)
        nc.vector.tensor_mul(o3[:], m3[:], rec3[:])
        nc.sync.dma_start(out_ap[:], o3[:])


# --------------------------------------------------------------------------
# Build + run
# --------------------------------------------------------------------------

_CACHE = {}


def build_nc():
    nc = bacc.Bacc(
        "TRN2", target_bir_lowering=False, debug=False, num_devices=NCORES
    )
    ins = {
        name: nc.dram_tensor(name, list(shape), dt, kind="ExternalInput").ap()
        for name, (shape, dt) in INPUT_SPECS.items()
    }
    out_ap = nc.dram_tensor("out", [1, KI], F32, kind="ExternalOutput").ap()
    with FastTailTileContext(nc) as tc:
        emit_kernel(tc, out_ap, ins)
    nc.compile()
    return nc


def kernel(**inputs):
    per_core = _host_prep(**inputs)
    if "nc" not in _CACHE:
        _CACHE["nc"] = build_nc()
    res = run_bass_kernel_spmd(
        _CACHE["nc"], per_core, core_ids=list(range(NCORES))
    )
    out = np.concatenate(
        [np.asarray(res.results[c]["out"], np.float32).reshape(-1)
         for c in range(NCORES)]
    )
    return out.reshape(1, 1, 10, 16, 1)


# revision 15
# speedup vs baseline: 1.3637x; 1.3637x over previous
"""Trainium2 Bass kernel for nn_CapsNet_69114613730132.

Strategy (8 NeuronCores, SPMD, zero collectives):
  The CapsNet routing loop is degenerate (self.bij is never updated, so
  cij stays 1/512) and collapses to: conv1 -> conv2 -> squash ->
  4096->160 matvec -> elementwise squash. The convolutions are tiny, so
  cross-core collectives (AllGather floor + a ~40us rank-alignment
  barrier measured on this fabric) cost more than replicating them.

  * Every core computes conv1 + conv2 (PrimaryCaps) + squash redundantly:
      conv1 as a 243-contraction matmul over a host-built im2col of x;
      conv2 as 81 (dy,dx) PSUM-accumulated matmuls over strided views of
      h (no im2col materialization), weights stationary, bf16.
  * The DigitCaps matvec output (160 = 10*16) is sharded 20-per-core via
    per-core weight slices => cores are fully independent; the host just
    concatenates the 8 (1,20) results. No communication at all.
  * All PE compute in bf16 (weights host-cast), f32 PSUM/vector math.

kernel(**inputs) takes the FULL unsharded inputs and returns the full
(1,1,10,16,1) float32 output.
"""
import numpy as np
import ml_dtypes

import concourse.bass as bass
import concourse.bacc as bacc
import concourse.tile as tile
import concourse.mybir as mybir
from concourse.bass_utils import run_bass_kernel_spmd
from concourse.tile import ScopedClock

FAST_TAIL = True


class FastTailTileContext(tile.TileContext):
    """TileContext tail without the second all-engine barrier.

    Tail = drain + barrier + sem clears. The drain has waited for every
    tracked semaphore target, so all sem-touching instructions retired
    before the barrier; the clears (on GpSimd) complete before GpSimd's
    own program end, and the next execution's NEFF entry barrier orders
    every engine after them — the trailing barrier is redundant."""

    def _drain_and_barrier(self, tick_clock, wait_clock):
        if not FAST_TAIL:
            return super()._drain_and_barrier(tick_clock, wait_clock)
        nc = self.nc
        drain_inst = nc.sync.drain()
        wait_clock.add_sem_waits(
            drain_inst.ins, ScopedClock({None: tick_clock.global_clock})
        )
        nc.all_engine_barrier()
        popped = nc._tile_sem_poison_stack.pop()
        assert popped is self._sem_poison
        nc.clear_and_free_semaphores(list(self.sems.allocated().values()))

BF16 = ml_dtypes.bfloat16
F32 = mybir.dt.float32
BF = mybir.dt.bfloat16

NCORES = 8
KI = 20             # digitcaps output elems per core (160 = 8*20)
# dydx positions per w2 DMA chunk. Tapered: big chunks amortize DMA issue
# overhead mid-stream; small tail chunks make the last completion
# semaphore fire right at stream end (in-flight transfers interleave at
# packet granularity, so a big last chunk's sem lags by the whole
# in-flight window).
W2CHUNKS = [12, 12, 12, 12, 12, 12, 5, 2, 2]
assert sum(W2CHUNKS) == 81


# --------------------------------------------------------------------------
# Host-side input marshalling (pure layout transforms + dtype casts)
# --------------------------------------------------------------------------

def _host_prep(x, conv_w, conv_b, pri_w, pri_b, W):
    x = np.asarray(x, np.float32)
    conv_w = np.asarray(conv_w, np.float32)
    conv_b = np.asarray(conv_b, np.float32)
    pri_w = np.asarray(pri_w, np.float32)
    pri_b = np.asarray(pri_b, np.float32)
    W = np.asarray(W, np.float32)

    # im2col of x: (243, 256), row (c,dy,dx), col (oy*16+ox)
    im2col1 = np.empty((3, 9, 9, 16, 16), np.float32)
    for dy in range(9):
        for dx in range(9):
            im2col1[:, dy, dx] = x[0, :, dy:dy + 16, dx:dx + 16]
    im2col1 = im2col1.reshape(243, 256).astype(BF16)

    W1T = conv_w.reshape(128, 243).T.astype(BF16)  # (243, 128)

    # (ic, dydx*256 + oc2) with oc2 = cap*8 + j
    w2s = (pri_w.reshape(256, 128, 9, 9)
           .transpose(2, 3, 1, 0)          # (dy, dx, ic, oc2)
           .reshape(81, 128, 256)
           .transpose(1, 0, 2)             # (ic, dydx, oc2)
           .reshape(128, 81 * 256).astype(BF16))

    # digitcaps weights V[h, s, p, ki]:
    #   oc2 = 128h+p; cap=oc2>>3; j=oc2&7; n = cap*16 + j*2 + (s>>3); jj = s&7
    Wd = W[0]  # (512, 10, 16, 8)
    oc2 = np.arange(256)
    n_base = (oc2 >> 3) * 16 + (oc2 & 7) * 2
    V = np.empty((2, 16, 128, 160), np.float32)
    for s in range(16):
        sel = Wd[n_base + (s >> 3), :, :, s & 7]      # (256, 10, 16)
        V[:, s] = sel.reshape(2, 128, 160)

    pb2 = pri_b.reshape(2, 128).T.copy()  # (128, 2) [p, h]
    cb2 = conv_b.reshape(128, 1)

    c1 = np.concatenate([im2col1, W1T], axis=1)  # (243, 384) = [im2col | w1t]
    shared = {
        "c1_a": np.ascontiguousarray(c1[:128]),
        "c1_b": np.ascontiguousarray(c1[128:]),
        "w2s": w2s,
        "cb": cb2,
        "pb": pb2,
    }
    per_core = []
    for c in range(NCORES):
        vsl = V[:, :, :, c * KI:(c + 1) * KI]                     # (2,16,128,20)
        vsl = vsl.transpose(2, 0, 1, 3).reshape(128, 32 * KI)     # (128, 640)
        d = dict(shared)
        d["v"] = np.ascontiguousarray(vsl).astype(BF16)
        per_core.append(d)
    return per_core


INPUT_SPECS = {
    "c1_a": ((128, 384), BF),
    "c1_b": ((115, 384), BF),
    "w2s": ((128, 81 * 256), BF),
    "v": ((128, 32 * KI), BF),
    "cb": ((128, 1), F32),
    "pb": ((128, 2), F32),
}


# --------------------------------------------------------------------------
# Device IR
# --------------------------------------------------------------------------

def emit_kernel(tc, out_ap, ins):
    nc = tc.nc
    with (
        tc.tile_pool(name="sb", bufs=1) as sb,
        tc.tile_pool(name="ps", bufs=1, space="PSUM") as ps,
    ):
        # ---- conv1 inputs first, then the w2 stream, on the sync HWDGE
        # ring (FIFO order).
        c1_a_sb = sb.tile([128, 384], BF)
        c1_b_sb = sb.tile([115, 384], BF)
        nc.sync.dma_start(c1_a_sb[:], ins["c1_a"][:])
        nc.sync.dma_start(c1_b_sb[:], ins["c1_b"][:])
        w2t = []
        off = 0
        for j, cn in enumerate(W2CHUNKS):
            wt = sb.tile([128, cn * 256], BF, name=f"w2t{j}")
            nc.sync.dma_start(
                wt[:], ins["w2s"][:, off * 256:(off + cn) * 256])
            w2t.append(wt)
            off += cn
        # small/late inputs on the scalar HWDGE ring (parallel)
        cb_sb = sb.tile([128, 1], F32)
        pb_sb = sb.tile([128, 2], F32)
        v_sb = sb.tile([128, 32 * KI], BF)
        for t, name in ((cb_sb, "cb"), (pb_sb, "pb"), (v_sb, "v")):
            nc.scalar.dma_start(t[:], ins[name][:])

        # ---- conv1: h = W1T.T @ im2col + conv_b  -> (128, 256) bf16
        psum1 = ps.tile([128, 256], F32)
        nc.tensor.matmul(psum1[:], c1_a_sb[:, 256:384], c1_a_sb[:, 0:256],
                         start=True, stop=False)
        nc.tensor.matmul(psum1[:], c1_b_sb[:, 256:384], c1_b_sb[:, 0:256],
                         start=False, stop=True)
        h_sb = sb.tile([128, 256], BF)
        nc.vector.tensor_scalar_add(h_sb[:], psum1[:], cb_sb[:])
        h4 = h_sb[:].rearrange("p (y x) -> p y x", y=16)

        # ---- conv2: 81 strided-view matmuls per oc2-half, PSUM-accumulated
        psum2a = ps.tile([128, 16], F32)
        psum2b = ps.tile([128, 16], F32)
        halves = (psum2a, psum2b)
        chunk_of = []
        for j, cn in enumerate(W2CHUNKS):
            chunk_of += [(j, k) for k in range(cn)]
        for dydx in range(81):
            dy, dx = divmod(dydx, 9)
            j, jj = chunk_of[dydx]
            rhs = h4[:, dy:dy + 8:2, dx:dx + 8:2]
            for hh in range(2):
                nc.tensor.matmul(
                    halves[hh][:],
                    w2t[j][:, jj * 256 + hh * 128: jj * 256 + (hh + 1) * 128],
                    rhs,
                    start=(dydx == 0), stop=(dydx == 80),
                )

        # ---- + pri_b -> x2b (128, 32) [p, h*16+s]
        x2b = sb.tile([128, 32], F32)
        for hh in range(2):
            nc.vector.tensor_scalar_add(
                x2b[:, hh * 16:(hh + 1) * 16], halves[hh][:], pb_sb[:, hh:hh + 1])

        # ---- squash factors per (p, h, s_hi) group of 8
        # f = sqrt(sq)/512 / (1+sq)   (1/512 cij folded in)
        t2 = sb.tile([128, 32], F32)
        nc.vector.tensor_mul(t2[:], x2b[:], x2b[:])
        sq = sb.tile([128, 4], F32)
        nc.vector.tensor_reduce(
            sq[:], t2[:].rearrange("p (g e) -> p g e", e=8),
            axis=mybir.AxisListType.X, op=mybir.AluOpType.add,
        )
        r_ = sb.tile([128, 4], F32)
        nc.scalar.activation(
            r_[:], sq[:], mybir.ActivationFunctionType.Sqrt,
            scale=1.0 / (512.0 * 512.0),
        )
        d2 = sb.tile([128, 4], F32)
        nc.vector.tensor_scalar_add(d2[:], sq[:], 1.0)
        rec2 = sb.tile([128, 4], F32)
        nc.vector.reciprocal(rec2[:], d2[:])
        f_ = sb.tile([128, 4], F32)
        nc.vector.tensor_mul(f_[:], r_[:], rec2[:])

        u_sb = sb.tile([128, 32], BF)
        nc.vector.tensor_mul(
            u_sb[:].rearrange("p (g e) -> p g e", e=8),
            x2b[:].rearrange("p (g e) -> p g e", e=8),
            f_[:].broadcast_to((128, 4, 8)),
        )

        # ---- digitcaps matvec: psum_d[0, ki] = sum_{h,s,p} u * V
        psum_d = ps.tile([1, KI], F32)
        for idx in range(32):
            nc.tensor.matmul(
                psum_d[:],
                u_sb[:, idx:idx + 1],
                v_sb[:, idx * KI:(idx + 1) * KI],
                start=(idx == 0), stop=(idx == 31),
            )

        # ---- final elementwise squash: vij = s*|s|/(1+s^2)
        s_sb = sb.tile([1, KI], F32)
        nc.vector.tensor_copy(s_sb[:], psum_d[:])
        t3 = sb.tile([1, KI], F32)
        nc.vector.tensor_mul(t3[:], s_sb[:], s_sb[:])
        d3 = sb.tile([1, KI], F32)
        nc.vector.tensor_scalar_add(d3[:], t3[:], 1.0)
        rec3 = sb.tile([1, KI], F32)
        nc.vector.reciprocal(rec3[:], d3[:])
        a3 = sb.tile([1, KI], F32)
        nc.scalar.activation(a3[:], t3[:], mybir.ActivationFunctionType.Sqrt)
        m3 = sb.tile([1, KI], F32)
        nc.vector.tensor_mul(m3[:], a3[:], s_sb[:])
        o3 = sb.tile([1, KI], F32)
        nc.vector.tensor_mul(o3[:], m3[:], rec3[:])
        nc.sync.dma_start(out_ap[:], o3[:])


# --------------------------------------------------------------------------
# Build + run
# --------------------------------------------------------------------------

_CACHE = {}


def build_nc():
    nc = bacc.Bacc(
        "TRN2", target_bir_lowering=False, debug=False, num_devices=NCORES
    )
    ins = {
        name: nc.dram_tensor(name, list(shape), dt, kind="ExternalInput").ap()
        for name, (shape, dt) in INPUT_SPECS.items()
    }
    out_ap = nc.dram_tensor("out", [1, KI], F32, kind="ExternalOutput").ap()
    with FastTailTileContext(nc) as tc:
        emit_kernel(tc, out_ap, ins)
    nc.compile()
    return nc


def kernel(**inputs):
    per_core = _host_prep(**inputs)
    if "nc" not in _CACHE:
        _CACHE["nc"] = build_nc()
    res = run_bass_kernel_spmd(
        _CACHE["nc"], per_core, core_ids=list(range(NCORES))
    )
    out = np.concatenate(
        [np.asarray(res.results[c]["out"], np.float32).reshape(-1)
         for c in range(NCORES)]
    )
    return out.reshape(1, 1, 10, 16, 1)
